# revision 3
# baseline (speedup 1.0000x reference)
"""Causal single-head attention (B=4, T=4096, C=1024, D=64) on 8 NeuronCores.

Sharding: core c = (batch b = c % 4, half h = c // 4).
Each core handles ALL queries of its batch, but only its half of the key
blocks (256-token key blocks with block index ≡ h mod 2).  This makes the
program identical on every core (pure SPMD, no control flow); cores differ
only in input data.  Each core emits unnormalized partial results
U^T = [V|1]^T @ exp(S^T) per query supertile; the host combines the two
halves per batch: O = (U0 + U1)[:64] / (U0 + U1)[64].

On-chip dataflow (all bf16 except PSUM/f32 accumulators):
  xq^T [C, T]   -> Q^T [64, T]          (matmul, C-tiled accumulation)
  xk^T [C, T/2] -> K^T, V^T [65, T/2]   (V^T row 64 = ones, for row-sums)
  V' [128, 65] per key tile              (PE transpose of V^T)
  S^T [128k, 512q] = K_tile @ Q^T        (matmul, contraction over D=64)
  P = exp(S^T/8) * causal_mask           (ACT exp from PSUM, DVE mask mul)
  U^T [65, 512] += V'_j^T @ P_j          (matmul, contraction over 128 keys)
"""
import sys
import numpy as np
import ml_dtypes

if "/opt/trn_rl_repo" not in sys.path:
    sys.path.insert(0, "/opt/trn_rl_repo")

import concourse.bacc as bacc
import concourse.mybir as mybir
from concourse import tile
from concourse import bass_utils

bf16 = mybir.dt.bfloat16
f32 = mybir.dt.float32
BF = ml_dtypes.bfloat16

B, T, C, D = 4, 4096, 1024, 64
NST = 8          # query supertiles per batch (512 queries each)
STQ = 512
TK = T // 2      # key tokens per core
NKT = TK // 128  # local 128-key tiles per core (16)
NC_ = C // 128   # 8 c-tiles

_CACHE = {}


def _build():
    nc = bacc.Bacc(None, target_bir_lowering=False, debug=False, num_devices=8)

    xq = nc.dram_tensor("xq", [C, T], bf16, kind="ExternalInput")
    xk = nc.dram_tensor("xk", [C, TK], bf16, kind="ExternalInput")
    w = nc.dram_tensor("w", [C, 192], bf16, kind="ExternalInput")   # Wq|Wk|Wv
    msk = nc.dram_tensor("msk", [256, STQ], bf16, kind="ExternalInput")
    idn = nc.dram_tensor("idn", [65, 65], bf16, kind="ExternalInput")
    out = nc.dram_tensor("out", [65, T], f32, kind="ExternalOutput")

    with tile.TileContext(nc) as tc:
        with tc.tile_pool(name="sb", bufs=1) as sb, \
             tc.tile_pool(name="pp", bufs=3) as pp, \
             tc.tile_pool(name="ps", bufs=2, space="PSUM") as ps:

            # ---- resident inputs ----
            xq_t = [sb.tile([128, T], bf16, tag=f"xq{c}", name=f"xq{c}")
                    for c in range(NC_)]
            xk_t = [sb.tile([128, TK], bf16, tag=f"xk{c}", name=f"xk{c}")
                    for c in range(NC_)]
            w_t = [sb.tile([128, 192], bf16, tag=f"w{c}", name=f"w{c}")
                   for c in range(NC_)]
            msk_t = sb.tile([128, 2 * STQ], bf16, tag="msk")
            idn_t = sb.tile([65, 65], bf16, tag="idn")
            for c in range(NC_):
                nc.sync.dma_start(xq_t[c][:], xq[128 * c:128 * (c + 1), :])
                nc.sync.dma_start(xk_t[c][:], xk[128 * c:128 * (c + 1), :])
                nc.sync.dma_start(w_t[c][:], w[128 * c:128 * (c + 1), :])
            nc.sync.dma_start(msk_t[:, 0:STQ], msk[0:128, :])
            nc.sync.dma_start(msk_t[:, STQ:2 * STQ], msk[128:256, :])
            nc.sync.dma_start(idn_t[:], idn[:])

            # ---- persistent intermediates ----
            qT = sb.tile([64, T], bf16, tag="qT")
            kT = sb.tile([64, TK], bf16, tag="kT")
            vT = sb.tile([65, TK], bf16, tag="vT")   # row 64 = ones
            vP = sb.tile([128, NKT * 65], bf16, tag="vP")  # V' tiles

            nc.vector.memset(vT[64:65, :], 1.0)

            # ---- projections ----
            # Q^T over all tokens, 512-wide chunks
            for st in range(NST):
                acc = ps.tile([64, STQ], f32, tag="work")
                for c in range(NC_):
                    nc.tensor.matmul(acc[:], w_t[c][:, 0:64],
                                     xq_t[c][:, STQ * st:STQ * (st + 1)],
                                     start=(c == 0), stop=(c == NC_ - 1))
                nc.vector.tensor_copy(qT[:, STQ * st:STQ * (st + 1)], acc[:])

            # K^T / V^T over local key tokens, 256-wide chunks
            for blk in range(TK // 256):
                sl = slice(256 * blk, 256 * (blk + 1))
                acck = ps.tile([64, 256], f32, tag="work")
                for c in range(NC_):
                    nc.tensor.matmul(acck[:], w_t[c][:, 64:128], xk_t[c][:, sl],
                                     start=(c == 0), stop=(c == NC_ - 1))
                nc.vector.tensor_copy(kT[:, sl], acck[:])
                accv = ps.tile([64, 256], f32, tag="work")
                for c in range(NC_):
                    nc.tensor.matmul(accv[:], w_t[c][:, 128:192], xk_t[c][:, sl],
                                     start=(c == 0), stop=(c == NC_ - 1))
                nc.vector.tensor_copy(vT[0:64, sl], accv[:])

            # V' tiles: transpose V^T (incl. ones row) per 128-key tile
            for j in range(NKT):
                tp = ps.tile([128, 65], bf16, tag="work")
                nc.tensor.transpose(tp[:], vT[:, 128 * j:128 * (j + 1)], idn_t[:])
                nc.vector.tensor_copy(vP[:, 65 * j:65 * (j + 1)], tp[:])

            # ---- attention ----
            for st in range(NST):
                qsl = slice(STQ * st, STQ * (st + 1))
                n = 2 * (st + 1)          # local key tiles for this supertile
                u = ps.tile([65, STQ], f32, tag="u")
                for j0 in range(0, n, 2):
                    s2 = ps.tile([128, 2 * STQ], f32, tag="s")
                    p2 = pp.tile([128, 2 * STQ], bf16, tag="p")
                    for d in range(2):
                        j = j0 + d
                        nc.tensor.matmul(s2[:, STQ * d:STQ * (d + 1)],
                                         kT[:, 128 * j:128 * (j + 1)],
                                         qT[:, qsl], start=True, stop=True)
                    nc.scalar.activation(p2[:], s2[:],
                                         mybir.ActivationFunctionType.Exp,
                                         scale=0.125)
                    if j0 == n - 2:  # diagonal pair -> causal masks
                        nc.vector.tensor_mul(p2[:], p2[:], msk_t[:])
                    for d in range(2):
                        j = j0 + d
                        nc.tensor.matmul(u[:], vP[:, 65 * j:65 * (j + 1)],
                                         p2[:, STQ * d:STQ * (d + 1)],
                                         start=(j == 0), stop=(j == n - 1))
                u_sb = pp.tile([65, STQ], f32, tag="u_sb")
                nc.vector.tensor_copy(u_sb[:], u[:])
                nc.sync.dma_start(out[:, qsl], u_sb[:])

    nc.compile()
    return nc


def _get_nc():
    if "nc" not in _CACHE:
        _CACHE["nc"] = _build()
    return _CACHE["nc"]


def kernel(x, Wq, Wk, Wv, _trace=False, _tmpdir=None):
    x = np.asarray(x)
    nc = _get_nc()

    xT = np.ascontiguousarray(x.transpose(0, 2, 1)).astype(BF)   # [B, C, T]
    w = np.concatenate([Wq, Wk, Wv], axis=1).astype(BF)          # [C, 192]
    idn = np.eye(65, dtype=BF)

    j = np.arange(128)[:, None]
    i = np.arange(STQ)[None, :]
    masks = {}
    for h in range(2):
        m0 = (j <= i - 256 * h).astype(BF)
        m1 = (j <= i - 256 * h - 128).astype(BF)
        masks[h] = np.concatenate([m0, m1], axis=0)

    # key-token selector: 256-blocks with block index ≡ h (mod 2)
    tok = np.arange(T)
    keysel = {h: ((tok // 256) % 2 == h) for h in range(2)}

    in_maps = []
    for c in range(8):
        b, h = c % 4, c // 4
        in_maps.append({
            "xq": xT[b],
            "xk": np.ascontiguousarray(xT[b][:, keysel[h]]),
            "w": w,
            "msk": masks[h],
            "idn": idn,
        })

    res = bass_utils.run_bass_kernel_spmd(nc, in_maps, core_ids=list(range(8)),
                                          trace=_trace, tmpdir=_tmpdir)
    _CACHE["last_results"] = res

    O = np.empty((B, T, D), dtype=np.float32)
    for b in range(B):
        U = res.results[b]["out"] + res.results[b + 4]["out"]    # [65, T]
        O[b] = (U[:D] / U[D:D + 1]).T
    return O



# revision 8
# speedup vs baseline: 1.2190x; 1.2190x over previous
"""Causal single-head attention (B=4, T=4096, C=1024, D=64) on 8 NeuronCores.

Sharding: core c = (batch b = c % 4, half h = c // 4).
Each core handles ALL queries of its batch, but only its half of the key
blocks (256-token key blocks with block index ≡ h mod 2).  Pure SPMD; cores
differ only in input data.  Each core emits unnormalized partial results
U^T = [V|1]^T @ exp(S^T) per query supertile; the host combines the two
halves per batch: O = (U0 + U1)[:64] / (U0 + U1)[64].

Performance structure:
  * x is re-laid-out on the host so one DMA per 512-query chunk brings all
    8 c-tiles ([128, 8, 512] AP) -- DMA dispatch on the sync engine costs
    ~600ns/instruction, so instruction count matters more than size.
  * Q projection uses duplicated weights [Wq|Wq] (M=128) -> qT2 holds Q^T
    in partitions 0-63 AND 64-127 (same cycles as M=64; matmul cost ~ N).
  * K/V projection fused as [Wv|Wk] (M=128): kvT partitions 0-63 = V^T
    (transpose-ready), 64-127 = K^T (odd-tile stationary, in place).
    K^T for even tiles is copied to kE partitions 0-63 by SBUF->SBUF DMA.
  * V' tiles are built by PE transpose of kvT[0:64] per 128-key tile into
    vP ([128, 65] per tile; col 64 = ones via one big memset).
  * S^T matmuls (contraction D=64 -> half the PE rows) are issued in
    row-tiled pairs: even key tile at tile_position (0,0), odd at (64,0).
    They execute concurrently in the PE array -> ~2x S throughput.
  * exp on ACT covers both PSUM banks of a pair in one instruction.
  * Output DMA is dispatched from the otherwise-idle GpSimd engine.
"""
import sys
import numpy as np
import ml_dtypes

if "/opt/trn_rl_repo" not in sys.path:
    sys.path.insert(0, "/opt/trn_rl_repo")

import concourse.bacc as bacc
import concourse.mybir as mybir
from concourse import tile
from concourse import bass_utils

bf16 = mybir.dt.bfloat16
f32 = mybir.dt.float32
BF = ml_dtypes.bfloat16

B, T, C, D = 4, 4096, 1024, 64
NST = 8          # query supertiles per batch (512 queries each)
STQ = 512
TK = T // 2      # key tokens per core
NKT = TK // 128  # local 128-key tiles per core (16)
NC_ = C // 128   # 8 c-tiles

_CACHE = {}


def _build():
    nc = bacc.Bacc(None, target_bir_lowering=False, debug=False, num_devices=8)

    # host layout: xq[p, c*T + t] = x^T[c*128 + p, t]  (c-major blocks)
    xq = nc.dram_tensor("xq", [128, NC_ * T], bf16, kind="ExternalInput")
    xk = nc.dram_tensor("xk", [128, NC_ * TK], bf16, kind="ExternalInput")
    w = nc.dram_tensor("w", [C, 256], bf16, kind="ExternalInput")  # Wq|Wq|Wv|Wk
    msk = nc.dram_tensor("msk", [256, STQ], bf16, kind="ExternalInput")
    idn = nc.dram_tensor("idn", [64, 64], bf16, kind="ExternalInput")
    out = nc.dram_tensor("out", [65, T], f32, kind="ExternalOutput")

    with tile.TileContext(nc) as tc:
        with tc.tile_pool(name="sb", bufs=1) as sb, \
             tc.tile_pool(name="pp", bufs=3) as pp, \
             tc.tile_pool(name="ps", bufs=2, space="PSUM") as ps:

            # ---- resident inputs ----
            xq_t = sb.tile([128, NC_ * T], bf16, tag="xq")
            xk_t = sb.tile([128, NC_ * TK], bf16, tag="xk")
            w_t = sb.tile([128, NC_ * 256], bf16, tag="w")
            msk_t = sb.tile([128, 2 * STQ], bf16, tag="msk")
            idn_t = sb.tile([64, 64], bf16, tag="idn")

            # single-instruction loads with 3D APs (c-dim folded into cols)
            nc.sync.dma_start(
                w_t[:].rearrange("p (a n) -> p a n", n=256),
                w[:].rearrange("(a p) n -> p a n", p=128))
            nc.sync.dma_start(
                msk_t[:].rearrange("p (a n) -> p a n", n=STQ),
                msk[:].rearrange("(a p) n -> p a n", p=128))
            nc.sync.dma_start(idn_t[:], idn[:])
            for st in range(NST):
                qsl = slice(STQ * st, STQ * (st + 1))
                ksl = slice(256 * st, 256 * (st + 1))
                nc.sync.dma_start(
                    xq_t[:].rearrange("p (a n) -> p a n", n=T)[:, :, qsl],
                    xq[:].rearrange("p (a n) -> p a n", n=T)[:, :, qsl])
                nc.sync.dma_start(
                    xk_t[:].rearrange("p (a n) -> p a n", n=TK)[:, :, ksl],
                    xk[:].rearrange("p (a n) -> p a n", n=TK)[:, :, ksl])

            # ---- persistent intermediates ----
            qT2 = sb.tile([128, T], bf16, tag="qT2")     # Q^T dup'd both halves
            kvT = sb.tile([128, TK], bf16, tag="kvT")    # p0:64 V^T, p64:128 K^T
            kE = sb.tile([128, TK], bf16, tag="kE")      # p0:64 = K^T (even tiles)
            vP = sb.tile([128, NKT * 65], bf16, tag="vP")  # V' tiles, col 64=ones

            nc.vector.memset(vP[:], 1.0)   # ones cols survive the transposes

            for st in range(NST):
                qsl = slice(STQ * st, STQ * (st + 1))
                ksl = slice(256 * st, 256 * (st + 1))

                # ---- Q projection (M=128: Wq|Wq) ----
                accq = ps.tile([128, STQ], f32, tag="acc")
                for c in range(NC_):
                    nc.tensor.matmul(accq[:], w_t[:, 256 * c:256 * c + 128],
                                     xq_t[:, T * c + STQ * st:T * c + STQ * (st + 1)],
                                     start=(c == 0), stop=(c == NC_ - 1))
                nc.vector.tensor_copy(qT2[:, qsl], accq[:])

                # ---- K/V projection (M=128: Wv|Wk) ----
                acckv = ps.tile([128, STQ], f32, tag="acc")
                for c in range(NC_):
                    nc.tensor.matmul(acckv[:, 0:256],
                                     w_t[:, 256 * c + 128:256 * (c + 1)],
                                     xk_t[:, TK * c + 256 * st:TK * c + 256 * (st + 1)],
                                     start=(c == 0), stop=(c == NC_ - 1))
                nc.vector.tensor_copy(kvT[:, ksl], acckv[:, 0:256])

                # K^T into partitions 0-63 for even row-tiles
                nc.sync.dma_start(kE[0:64, ksl], kvT[64:128, ksl])
                # V' tiles via PE transpose: [64,128] -> [128,64]
                for dj in range(2):
                    j = 2 * st + dj
                    tp = ps.tile([128, 64], bf16, tag="tp", bufs=1)
                    nc.tensor.transpose(tp[:], kvT[0:64, 128 * j:128 * (j + 1)],
                                        idn_t[:])
                    nc.vector.tensor_copy(vP[:, 65 * j:65 * j + 64], tp[:])

                # ---- attention for supertile st ----
                n = 2 * (st + 1)          # local key tiles for this supertile
                u = ps.tile([65, STQ], f32, tag="u", bufs=1)
                for j0 in range(0, n, 2):
                    s2 = ps.tile([128, 2 * STQ], f32, tag="s")
                    p2 = pp.tile([128, 2 * STQ], bf16, tag="p")
                    # row-tiled S pair: even tile rows 0-63, odd rows 64-127
                    nc.tensor.matmul(s2[:, 0:STQ],
                                     kE[0:64, 128 * j0:128 * (j0 + 1)],
                                     qT2[0:64, qsl], start=True, stop=True)
                    nc.tensor.matmul(s2[:, STQ:2 * STQ],
                                     kvT[64:128, 128 * (j0 + 1):128 * (j0 + 2)],
                                     qT2[64:128, qsl], start=True, stop=True)
                    nc.scalar.activation(p2[:], s2[:],
                                         mybir.ActivationFunctionType.Exp,
                                         scale=0.125)
                    if j0 == n - 2:  # diagonal pair -> causal masks
                        nc.vector.tensor_mul(p2[:], p2[:], msk_t[:])
                    for dj in range(2):
                        j = j0 + dj
                        nc.tensor.matmul(u[:], vP[:, 65 * j:65 * (j + 1)],
                                         p2[:, STQ * dj:STQ * (dj + 1)],
                                         start=(j == 0), stop=(j == n - 1))
                u_sb = pp.tile([65, STQ], f32, tag="u_sb")
                nc.vector.tensor_copy(u_sb[:], u[:])
                nc.gpsimd.dma_start(out[:, qsl], u_sb[:])

    nc.compile()
    return nc


def _get_nc():
    if "nc" not in _CACHE:
        _CACHE["nc"] = _build()
    return _CACHE["nc"]


def kernel(x, Wq, Wk, Wv, _trace=False, _tmpdir=None):
    x = np.asarray(x)
    nc = _get_nc()

    xT = np.ascontiguousarray(x.transpose(0, 2, 1)).astype(BF)   # [B, C, T]
    w = np.concatenate([Wq, Wq, Wv, Wk], axis=1).astype(BF)      # [C, 256]
    idn = np.eye(64, dtype=BF)

    j = np.arange(128)[:, None]
    i = np.arange(STQ)[None, :]
    masks = {}
    for h in range(2):
        m0 = (j <= i - 256 * h).astype(BF)
        m1 = (j <= i - 256 * h - 128).astype(BF)
        masks[h] = np.concatenate([m0, m1], axis=0)

    # key-token selector: 256-blocks with block index ≡ h (mod 2)
    tok = np.arange(T)
    keysel = {h: ((tok // 256) % 2 == h) for h in range(2)}

    in_maps = []
    for c in range(8):
        b, h = c % 4, c // 4
        xq_r = xT[b].reshape(NC_, 128, T).transpose(1, 0, 2).reshape(128, NC_ * T)
        xk_full = xT[b][:, keysel[h]]
        xk_r = xk_full.reshape(NC_, 128, TK).transpose(1, 0, 2).reshape(128, NC_ * TK)
        in_maps.append({
            "xq": np.ascontiguousarray(xq_r),
            "xk": np.ascontiguousarray(xk_r),
            "w": w,
            "msk": masks[h],
            "idn": idn,
        })

    res = bass_utils.run_bass_kernel_spmd(nc, in_maps, core_ids=list(range(8)),
                                          trace=_trace, tmpdir=_tmpdir)
    _CACHE["last_results"] = res

    O = np.empty((B, T, D), dtype=np.float32)
    for b in range(B):
        U = res.results[b]["out"] + res.results[b + 4]["out"]    # [65, T]
        O[b] = (U[:D] / U[D:D + 1]).T
    return O


# revision 13
# speedup vs baseline: 1.2883x; 1.0569x over previous
"""Causal single-head attention (B=4, T=4096, C=1024, D=64) on 8 NeuronCores.

Sharding: core c = (batch b = c % 4, half h = c // 4).
Each core handles ALL queries of its batch, but only its half of the key
blocks (256-token key blocks with block index ≡ h mod 2).  Pure SPMD; cores
differ only in input data.  Each core emits unnormalized partial results
U^T = [V|1]^T @ exp(S^T) per query supertile; the host combines the two
halves per batch: O = (U0 + U1)[:64] / (U0 + U1)[64].

Performance structure:
  * x is re-laid-out on the host so one DMA per 512-query chunk brings all
    8 c-tiles ([128, 8, 512] AP) -- DMA dispatch on the sync engine costs
    ~600ns/instruction, so instruction count matters more than size.
  * Q projection uses duplicated weights [Wq|Wq] (M=128) -> qT2 holds Q^T
    in partitions 0-63 AND 64-127 (same cycles as M=64; matmul cost ~ N).
  * K/V projection fused as [Wv|Wk] (M=128): kvT partitions 0-63 = V^T
    (transpose-ready), 64-127 = K^T (odd-tile stationary, in place).
    K^T for even tiles is copied to kE partitions 0-63 by SBUF->SBUF DMA.
  * V' tiles are built by PE transpose of kvT[0:64] per 128-key tile into
    vP ([128, 65] per tile; col 64 = ones via one big memset).
  * S^T matmuls (contraction D=64 -> half the PE rows) are issued in
    row-tiled pairs: even key tile at tile_position (0,0), odd at (64,0).
    They execute concurrently in the PE array -> ~2x S throughput.
  * exp on ACT covers both PSUM banks of a pair in one instruction.
  * Output DMA is dispatched from the otherwise-idle GpSimd engine.
"""
import sys
import numpy as np
import ml_dtypes

if "/opt/trn_rl_repo" not in sys.path:
    sys.path.insert(0, "/opt/trn_rl_repo")

import concourse.bacc as bacc
import concourse.mybir as mybir
from concourse import tile
from concourse import bass_utils

bf16 = mybir.dt.bfloat16
f32 = mybir.dt.float32
BF = ml_dtypes.bfloat16

B, T, C, D = 4, 4096, 1024, 64
NST = 8          # query supertiles per batch (512 queries each)
STQ = 512
TK = T // 2      # key tokens per core
NKT = TK // 128  # local 128-key tiles per core (16)
NC_ = C // 128   # 8 c-tiles

_CACHE = {}


def _build():
    nc = bacc.Bacc(None, target_bir_lowering=False, debug=False, num_devices=8)

    # host layout (st-major, fully contiguous per 512-query chunk):
    #   xq[p, 4096*st + 512*c + t'] = x^T[c*128 + p, 512*st + t']
    #   xk[p, 2048*st + 256*c + t'] = xk_full^T[c*128 + p, 256*st + t']
    xq = nc.dram_tensor("xq", [128, NC_ * T], bf16, kind="ExternalInput")
    xk = nc.dram_tensor("xk", [128, NC_ * TK], bf16, kind="ExternalInput")
    w = nc.dram_tensor("w", [C, 256], bf16, kind="ExternalInput")  # Wq|Wq|Wv|Wk
    msk = nc.dram_tensor("msk", [256, STQ], bf16, kind="ExternalInput")
    idn = nc.dram_tensor("idn", [64, 64], bf16, kind="ExternalInput")
    out = nc.dram_tensor("out", [65, T], f32, kind="ExternalOutput")

    with tile.TileContext(nc) as tc:
        with tc.tile_pool(name="sb", bufs=1) as sb, \
             tc.tile_pool(name="pp", bufs=3) as pp, \
             tc.tile_pool(name="ps", bufs=2, space="PSUM") as ps:

            # ---- resident inputs ----
            xq_t = sb.tile([128, NC_ * T], bf16, tag="xq")
            xk_t = sb.tile([128, NC_ * TK], bf16, tag="xk")
            w_t = sb.tile([128, NC_ * 256], bf16, tag="w")
            msk_t = sb.tile([128, 2 * STQ], bf16, tag="msk")
            idn_t = sb.tile([64, 64], bf16, tag="idn")

            # single-instruction loads with 3D APs (c-dim folded into cols)
            nc.sync.dma_start(
                w_t[:].rearrange("p (a n) -> p a n", n=256),
                w[:].rearrange("(a p) n -> p a n", p=128))
            nc.sync.dma_start(
                msk_t[:].rearrange("p (a n) -> p a n", n=STQ),
                msk[:].rearrange("(a p) n -> p a n", p=128))
            nc.sync.dma_start(idn_t[:], idn[:])
            for st in range(NST):
                xsl = slice(8 * STQ * st, 8 * STQ * (st + 1))
                ksl8 = slice(8 * 256 * st, 8 * 256 * (st + 1))
                nc.sync.dma_start(xq_t[:, xsl], xq[:, xsl])
                nc.scalar.dma_start(xk_t[:, ksl8], xk[:, ksl8])

            # ---- persistent intermediates ----
            qT2 = sb.tile([128, T], bf16, tag="qT2")     # Q^T dup'd both halves
            kvT = sb.tile([128, TK], bf16, tag="kvT")    # p0:64 V^T, p64:128 K^T
            kE = sb.tile([128, TK], bf16, tag="kE")      # p0:64 = K^T (even tiles)
            vP = sb.tile([128, NKT * 65], bf16, tag="vP")  # V' tiles, col 64=ones

            nc.vector.memset(vP[:], 1.0)   # ones cols survive the transposes

            for st in range(NST):
                qsl = slice(STQ * st, STQ * (st + 1))
                ksl = slice(256 * st, 256 * (st + 1))

                # ---- Q projection (M=128: Wq|Wq) ----
                accq = ps.tile([128, STQ], f32, tag="acc")
                for c in range(NC_):
                    xo = 8 * STQ * st + STQ * c
                    nc.tensor.matmul(accq[:], w_t[:, 256 * c:256 * c + 128],
                                     xq_t[:, xo:xo + STQ],
                                     start=(c == 0), stop=(c == NC_ - 1))
                nc.vector.tensor_copy(qT2[:, qsl], accq[:])

                # ---- K/V projection (M=128: Wv|Wk) ----
                acckv = ps.tile([128, STQ], f32, tag="acc")
                for c in range(NC_):
                    ko = 8 * 256 * st + 256 * c
                    nc.tensor.matmul(acckv[:, 0:256],
                                     w_t[:, 256 * c + 128:256 * (c + 1)],
                                     xk_t[:, ko:ko + 256],
                                     start=(c == 0), stop=(c == NC_ - 1))
                nc.vector.tensor_copy(kvT[:, ksl], acckv[:, 0:256])

                # K^T into partitions 0-63 for even row-tiles
                nc.sync.dma_start(kE[0:64, ksl], kvT[64:128, ksl])
                # V' tiles via PE transpose: [64,128] -> [128,64]
                for dj in range(2):
                    j = 2 * st + dj
                    tp = ps.tile([128, 64], bf16, tag="tp", bufs=1)
                    nc.tensor.transpose(tp[:], kvT[0:64, 128 * j:128 * (j + 1)],
                                        idn_t[:])
                    nc.vector.tensor_copy(vP[:, 65 * j:65 * j + 64], tp[:])

                # ---- attention for supertile st ----
                n = 2 * (st + 1)          # local key tiles for this supertile
                u = ps.tile([65, STQ], f32, tag="u", bufs=1)
                for j0 in range(0, n, 2):
                    s2 = ps.tile([128, 2 * STQ], f32, tag="s")
                    p2 = pp.tile([128, 2 * STQ], bf16, tag="p")
                    # row-tiled S pair: even tile rows 0-63, odd rows 64-127
                    nc.tensor.matmul(s2[:, 0:STQ],
                                     kE[0:64, 128 * j0:128 * (j0 + 1)],
                                     qT2[0:64, qsl], start=True, stop=True)
                    nc.tensor.matmul(s2[:, STQ:2 * STQ],
                                     kvT[64:128, 128 * (j0 + 1):128 * (j0 + 2)],
                                     qT2[64:128, qsl], start=True, stop=True)
                    nc.scalar.activation(p2[:], s2[:],
                                         mybir.ActivationFunctionType.Exp,
                                         scale=0.125)
                    if j0 == n - 2:  # diagonal pair -> causal masks
                        nc.vector.tensor_mul(p2[:], p2[:], msk_t[:])
                    for dj in range(2):
                        j = j0 + dj
                        nc.tensor.matmul(u[:], vP[:, 65 * j:65 * (j + 1)],
                                         p2[:, STQ * dj:STQ * (dj + 1)],
                                         start=(j == 0), stop=(j == n - 1))
                u_sb = pp.tile([65, STQ], f32, tag="u_sb")
                nc.vector.tensor_copy(u_sb[:], u[:])
                nc.gpsimd.dma_start(out[:, qsl], u_sb[:])

    nc.compile()
    return nc


def _get_nc():
    if "nc" not in _CACHE:
        _CACHE["nc"] = _build()
    return _CACHE["nc"]


def kernel(x, Wq, Wk, Wv, _trace=False, _tmpdir=None):
    x = np.asarray(x)
    nc = _get_nc()

    xT = np.ascontiguousarray(x.transpose(0, 2, 1)).astype(BF)   # [B, C, T]
    w = np.concatenate([Wq, Wq, Wv, Wk], axis=1).astype(BF)      # [C, 256]
    idn = np.eye(64, dtype=BF)

    j = np.arange(128)[:, None]
    i = np.arange(STQ)[None, :]
    masks = {}
    for h in range(2):
        m0 = (j <= i - 256 * h).astype(BF)
        m1 = (j <= i - 256 * h - 128).astype(BF)
        masks[h] = np.concatenate([m0, m1], axis=0)

    # key-token selector: 256-blocks with block index ≡ h (mod 2)
    tok = np.arange(T)
    keysel = {h: ((tok // 256) % 2 == h) for h in range(2)}

    in_maps = []
    for c in range(8):
        b, h = c % 4, c // 4
        # st-major contiguous: xq_r[p, 4096*st + 512*c + t']
        xq_r = (xT[b].reshape(NC_, 128, NST, STQ)
                .transpose(1, 2, 0, 3).reshape(128, NC_ * T))
        xk_full = xT[b][:, keysel[h]]
        xk_r = (xk_full.reshape(NC_, 128, NST, 256)
                .transpose(1, 2, 0, 3).reshape(128, NC_ * TK))
        in_maps.append({
            "xq": np.ascontiguousarray(xq_r),
            "xk": np.ascontiguousarray(xk_r),
            "w": w,
            "msk": masks[h],
            "idn": idn,
        })

    res = bass_utils.run_bass_kernel_spmd(nc, in_maps, core_ids=list(range(8)),
                                          trace=_trace, tmpdir=_tmpdir)
    _CACHE["last_results"] = res

    O = np.empty((B, T, D), dtype=np.float32)
    for b in range(B):
        U = res.results[b]["out"] + res.results[b + 4]["out"]    # [65, T]
        O[b] = (U[:D] / U[D:D + 1]).T
    return O
